# revision 1
# baseline (speedup 1.0000x reference)
"""DeepFM (nn_DeepFM_77558519431939) Trainium2 Bass kernel.

Strategy (8 NeuronCores, SPMD, no collectives):
  - Replicate the embedding tables on every core; data-parallel the batch
    (16384 samples -> 2048 per core).
  - The gather is descriptor-generation bound on the GpSimd SWDGE
    (~1.4us per instruction nearly independent of offset count, 128
    offsets max per indirect instruction), so:
      * fields 0-4 (vocab > 32767): 80 indirect_dma_start instructions
        (16 sample-blocks x 5 fields, 128 rows each) against a bf16
        augmented table tb [S, 12] (10 emb dims + w_first + pad).
      * fields 5-7 (vocab <= 32767): int16-indexed dma_gather ucode in
        transpose mode (4 instructions of 512 idxs per field) against
        per-field bf16 tables [V, 128] (256B rows).  Output is directly
        feature-major [128 elems, samples]; rows 0..11 are then moved
        into X with partition-shifted SBUF->SBUF DMAs on the sync
        engine (off the gpsimd critical path).
  - X [104, 2048] bf16, feature-major:
        rows f*12+e (e<10): emb dim e of field f; f*12+10: w_first;
        f*12+11: pad; 96..102: dense features; 103: const 1.0
  - Head per 128-sample block (all weights folded on host, bf16
    matmuls with fp32 PSUM accumulation):
        H1 = relu(W1s^T X)          (dense-proj + b1 folded into W1s)
        H2 = relu(W2^T H1 + b2)
        SD = SDw^T X                (rows 0..9 = s, 10..19 = dense_emb)
        XSQ = X[0:96]^2, SD2 = SD^2
        FIN = a1^T X + esq^T XSQ + es2^T SD2 + W3^T H2
        out = sigmoid(FIN)
"""

from contextlib import ExitStack

import numpy as np
import ml_dtypes

import concourse.bass as bass
import concourse.bacc as bacc
import concourse.mybir as mybir
import concourse.tile as tile
from concourse import library_config

# ---- problem constants (hardcoded; must match the reference) ----
VOCABS = [1000000, 500000, 200000, 100000, 50000, 10000, 5000, 1000]
S = int(np.sum(VOCABS))  # 1,866,000
OFFSETS = np.concatenate([[0], np.cumsum(VOCABS)[:-1]]).astype(np.int64)
B = 16384
EMB = 10
N_DENSE = 7
F = len(VOCABS)      # 8
FB = 5               # big fields (indirect path)
FS = 3               # small fields (dma_gather path)
HID = 128

N_CORES = 8
BL = B // N_CORES    # 2048 per core
RW = 12              # augmented row width (10 emb + wf + pad)
KX = 104             # X rows: 96 gathered + 7 dense + 1 const
NBLK = BL // 128     # 16 sample blocks of 128
DGI = 4              # dma_gather instructions per small field (512 idx each;
                     # transpose mode emits 2 descs/idx, ring caps at 1024)

F32 = mybir.dt.float32
BF16 = mybir.dt.bfloat16
I32 = mybir.dt.int32
I16 = mybir.dt.int16

_cached = {}


def _build_program(debug_dump=False):
    """Build the SPMD Bass program (same for all cores)."""
    nc = bacc.Bacc("TRN2", target_bir_lowering=False, debug=False)

    tb_d = nc.dram_tensor("tb", [S, RW], BF16, kind="ExternalInput").ap()
    ts_d = [nc.dram_tensor(f"ts{q}", [VOCABS[FB + q], 128], BF16,
                           kind="ExternalInput").ap() for q in range(FS)]
    idx_d = nc.dram_tensor("idxs", [128, NBLK * FB], I32,
                           kind="ExternalInput").ap()
    ixw_d = nc.dram_tensor("ixw", [128, FS * (BL // 16)], I16,
                           kind="ExternalInput").ap()
    dn8_d = nc.dram_tensor("dn8", [8, BL], BF16, kind="ExternalInput").ap()
    # all small weights packed into one bf16 tensor:
    # cols: idn 0:128 | w1s 128:256 | w2 256:384 | sdw 385:405 |
    #       a1 405 | esq 406 | es2 407 | w3 408   (col 384 unused)
    wpk_d = nc.dram_tensor("wpk", [128, 409], BF16, kind="ExternalInput").ap()
    b2c_d = nc.dram_tensor("b2c", [128, 1], F32, kind="ExternalInput").ap()
    out_d = nc.dram_tensor("out", [1, BL], F32, kind="ExternalOutput").ap()
    if debug_dump:
        xdmp_d = nc.dram_tensor("xdmp", [KX, BL], BF16,
                                kind="ExternalOutput").ap()

    with ExitStack() as ctx:
        tc = ctx.enter_context(tile.TileContext(nc))
        const = ctx.enter_context(tc.tile_pool(name="const", bufs=1))
        gpool = ctx.enter_context(tc.tile_pool(name="gch", bufs=NBLK))
        hpool = ctx.enter_context(tc.tile_pool(name="h", bufs=2))
        qpool = ctx.enter_context(tc.tile_pool(name="xsq", bufs=2))
        pp_x = ctx.enter_context(tc.tile_pool(name="ppx", bufs=2, space="PSUM"))
        pp_h = ctx.enter_context(tc.tile_pool(name="pph", bufs=2, space="PSUM"))
        pp_s = ctx.enter_context(tc.tile_pool(name="pps", bufs=2, space="PSUM"))
        pp_f = ctx.enter_context(tc.tile_pool(name="ppf", bufs=2, space="PSUM"))

        nc.gpsimd.load_library(library_config.mlp)

        # index tiles first: the gathers depend only on them
        idx_t = const.tile([128, NBLK * FB], I32)
        nc.sync.dma_start(idx_t[:], idx_d[:])
        ixw_t = const.tile([128, FS * (BL // 16)], I16)
        nc.sync.dma_start(ixw_t[:], ixw_d[:])

        # non-gather-critical uploads go via the scalar/vector HWDGE so the
        # gpsimd gather stream only waits on the sync-engine idx uploads.
        wpk_t = const.tile([128, 409], BF16)
        nc.scalar.dma_start(wpk_t[:], wpk_d[:])
        b2_t = const.tile([128, 1], F32)
        nc.scalar.dma_start(b2_t[:], b2c_d[:])
        idn_t = wpk_t[:, 0:128]
        w1s_t = wpk_t[0:KX, 128:256]
        w2_t = wpk_t[:, 256:384]
        sdw_t = wpk_t[0:KX, 385:405]
        a1_t = wpk_t[0:KX, 405:406]
        esq_t = wpk_t[0:96, 406:407]
        es2_t = wpk_t[0:20, 407:408]
        w3_t = wpk_t[:, 408:409]

        # X: feature-major activations (bf16)
        x_t = const.tile([KX, BL], BF16)
        nc.scalar.dma_start(x_t[96:104, :], dn8_d[:])

        # small-field transpose-gather destinations [128 elems, samples]
        gs = []
        for q in range(FS):
            g_small = const.tile([128, 1, BL], BF16, name=f"gs{q}")
            gs.append(g_small)

        out_sb = const.tile([1, BL], F32)

        RELU = mybir.ActivationFunctionType.Relu
        SQUARE = mybir.ActivationFunctionType.Square
        SIGMOID = mybir.ActivationFunctionType.Sigmoid

        WPF = BL // 16  # idx words per small field in ixw

        def dg_small(q, k):
            """dma_gather instr k (of DGI) for small field q."""
            n = BL // DGI
            nc.gpsimd.dma_gather(
                gs[q][:, :, n * k:n * (k + 1)], ts_d[q][:],
                ixw_t[:, WPF * q + (n // 16) * k: WPF * q + (n // 16) * (k + 1)],
                n, n, 128, transpose=True, single_packet=False)

        def sb_copy(q, k):
            """move small field q's rows into X for column half k."""
            n = BL // DGI
            cols = slice(n * k, n * (k + 1))
            nc.sync.dma_start(x_t[60 + 12 * q:72 + 12 * q, cols],
                              gs[q][0:12, 0, cols])

        gball = const.tile([128, NBLK, FB, RW], BF16)

        def gather_block(j):
            for f in range(FB):
                nc.gpsimd.indirect_dma_start(
                    out=gball[:, j, f, :],
                    out_offset=None,
                    in_=tb_d[:],
                    in_offset=bass.IndirectOffsetOnAxis(
                        ap=idx_t[:, j * FB + f:j * FB + f + 1], axis=0
                    ),
                )
            return gball[:, j, :, :]

        def compute_block(j, gb):
            cols = slice(128 * j, 128 * (j + 1))
            xp = pp_x.tile([60, 128], BF16, tag="xp")
            nc.tensor.transpose(out=xp[:], in_=gb[:], identity=idn_t)
            nc.vector.tensor_copy(x_t[0:60, cols], xp[:])

            # MLP
            h1p = pp_h.tile([HID, 128], F32, tag="hp")
            nc.tensor.matmul(out=h1p[:], lhsT=w1s_t, rhs=x_t[:, cols],
                             start=True, stop=True)
            h1_t = hpool.tile([HID, 128], BF16, tag="h")
            nc.scalar.activation(h1_t[:], h1p[:], RELU)
            h2p = pp_h.tile([HID, 128], F32, tag="hp")
            nc.tensor.matmul(out=h2p[:], lhsT=w2_t, rhs=h1_t[:],
                             start=True, stop=True)
            h2_t = hpool.tile([HID, 128], BF16, tag="h")
            nc.scalar.activation(h2_t[:], h2p[:], RELU, bias=b2_t)

            # s / dense_emb rows
            sdp = pp_s.tile([20, 128], F32, tag="sd")
            nc.tensor.matmul(out=sdp[:], lhsT=sdw_t, rhs=x_t[:, cols],
                             start=True, stop=True)

            xsq = qpool.tile([96, 128], BF16, tag="xsq")
            nc.vector.tensor_mul(xsq[:], x_t[0:96, cols], x_t[0:96, cols])
            sd2 = qpool.tile([20, 128], BF16, tag="sd2")
            nc.scalar.activation(sd2[:], sdp[:], SQUARE)

            # final accumulation + sigmoid
            fin = pp_f.tile([1, 128], F32, tag="fin")
            nc.tensor.matmul(out=fin[:], lhsT=a1_t, rhs=x_t[:, cols],
                             start=True, stop=False)
            nc.tensor.matmul(out=fin[:], lhsT=esq_t, rhs=xsq[:],
                             start=False, stop=False)
            nc.tensor.matmul(out=fin[:], lhsT=es2_t, rhs=sd2[:],
                             start=False, stop=False)
            nc.tensor.matmul(out=fin[:], lhsT=w3_t, rhs=h2_t[:],
                             start=False, stop=True)
            nc.scalar.activation(out_sb[:, cols], fin[:], SIGMOID)

        # gpsimd stream: per quarter, indirects first then the 3 small-field
        # dma_gathers -- the ~9us mlp-ucode init after load_library only
        # blocks the first dma_gather, so leading with indirects hides it.
        # Last quarter flips (dgt first) to keep the tail to one block chain.
        QB = NBLK // DGI
        gbs = {}
        for k in range(DGI):
            last = k == DGI - 1
            if last:
                for q in range(FS):
                    dg_small(q, k)
                for q in range(FS):
                    sb_copy(q, k)
            for j in range(QB * k, QB * (k + 1)):
                gbs[j] = gather_block(j)
            if not last:
                for q in range(FS):
                    dg_small(q, k)
                for q in range(FS):
                    sb_copy(q, k)
            for j in range(QB * k, QB * (k + 1)):
                compute_block(j, gbs[j])
            qc = slice((BL // DGI) * k, (BL // DGI) * (k + 1))
            nc.sync.dma_start(out_d[:, qc], out_sb[:, qc])
        if debug_dump:
            nc.sync.dma_start(xdmp_d[:], x_t[:])

    nc.compile()
    return nc


def _host_prep(sparse_feature, dense_feature, emb_table, W_dense, b_dense,
               w_first, b_first, W1, b1, W2, b2, W3, b3):
    """Build the augmented tables, folded weights, and per-core in_maps."""
    f32 = np.float32
    bf16 = ml_dtypes.bfloat16
    emb_table = np.asarray(emb_table, dtype=f32)
    W_dense = np.asarray(W_dense, dtype=f32)      # [10, 7]
    b_dense = np.asarray(b_dense, dtype=f32)      # [10]
    w_first = np.asarray(w_first, dtype=f32)      # [S+7]
    b_first = np.asarray(b_first, dtype=f32)      # [1]
    W1 = np.asarray(W1, dtype=f32)                # [90, 128]
    b1 = np.asarray(b1, dtype=f32)                # [128]
    W2 = np.asarray(W2, dtype=f32)                # [128, 128]
    b2 = np.asarray(b2, dtype=f32)                # [128]
    W3 = np.asarray(W3, dtype=f32)                # [128, 1]
    b3 = np.asarray(b3, dtype=f32)                # [1]

    tb = np.zeros((S, RW), dtype=bf16)
    tb[:, :EMB] = emb_table
    tb[:, EMB] = w_first[:S]

    ts = []
    for q in range(FS):
        f = FB + q
        v = VOCABS[f]
        o = int(OFFSETS[f])
        t = np.zeros((v, 128), dtype=bf16)
        t[:, :EMB] = emb_table[o:o + v]
        t[:, EMB] = w_first[o:o + v]
        ts.append(t)

    w1s = np.zeros((KX, HID), dtype=f32)
    for f in range(F):
        w1s[f * RW:f * RW + EMB] = W1[f * EMB:(f + 1) * EMB]
    w1s[96:103] = W_dense.T @ W1[F * EMB:]               # [7,128]
    w1s[103] = b1 + b_dense @ W1[F * EMB:]

    sdw = np.zeros((KX, 20), dtype=f32)
    for f in range(F):
        for e in range(EMB):
            sdw[f * RW + e, e] = 1.0
    sdw[96:103, 0:10] = W_dense.T
    sdw[103, 0:10] = b_dense
    sdw[96:103, 10:20] = W_dense.T
    sdw[103, 10:20] = b_dense

    a1 = np.zeros((KX, 1), dtype=f32)
    for f in range(F):
        a1[f * RW + EMB] = 1.0
    a1[96:103, 0] = w_first[S:]
    a1[103] = b_first[0] + b3[0]

    esq = np.zeros((96, 1), dtype=f32)
    for f in range(F):
        esq[f * RW:f * RW + EMB] = -0.5
    es2 = np.zeros((20, 1), dtype=f32)
    es2[0:10] = 0.5
    es2[10:20] = -0.5

    idx_g = (np.asarray(sparse_feature, dtype=np.int64)
             + OFFSETS[None, :]).astype(np.int64)          # [B, F]
    dense = np.asarray(dense_feature, dtype=f32)           # [B, 7]

    wpk = np.zeros((128, 409), dtype=bf16)
    wpk[:, 0:128] = np.eye(128, dtype=f32)
    wpk[0:KX, 128:256] = w1s
    wpk[:, 256:384] = W2
    wpk[0:KX, 385:405] = sdw
    wpk[0:KX, 405] = a1[:, 0]
    wpk[0:96, 406] = esq[:, 0]
    wpk[0:20, 407] = es2[:, 0]
    wpk[:, 408] = W3.reshape(HID)
    b2c = b2.reshape(128, 1).astype(f32)

    common = {"tb": tb, "wpk": wpk, "b2c": b2c}
    for q in range(FS):
        common[f"ts{q}"] = ts[q]

    WPF = BL // 16
    in_maps = []
    for c in range(N_CORES):
        lo, hi = c * BL, (c + 1) * BL
        # big fields: global ids, [128, NBLK*FB], col j*FB+f
        lg = idx_g[lo:hi, :FB].reshape(NBLK, 128, FB).astype(np.int32)
        idxs = np.ascontiguousarray(
            lg.transpose(1, 0, 2).reshape(128, NBLK * FB))
        # small fields: local ids wrapped in 16 partitions, replicated x8
        ixw = np.zeros((128, FS * WPF), dtype=np.int16)
        ar = np.arange(BL)
        for q in range(FS):
            loc = (np.asarray(sparse_feature[lo:hi, FB + q], dtype=np.int64)
                   ).astype(np.int16)
            ixw[ar % 16, WPF * q + ar // 16] = loc
        for g in range(1, 8):
            ixw[16 * g:16 * (g + 1)] = ixw[0:16]
        dn8 = np.ones((8, BL), dtype=bf16)
        dn8[:7] = dense[lo:hi].T
        in_maps.append(dict(common, idxs=idxs, ixw=ixw, dn8=dn8))
    return in_maps


def _get_program(debug_dump=False):
    key = ("nc", debug_dump)
    if key not in _cached:
        _cached[key] = _build_program(debug_dump)
    return _cached[key]


def run_on_device(in_maps, trace=False, debug_dump=False):
    """Run the SPMD program on 8 NeuronCores.  Returns (results, exec_ns)."""
    from concourse.bass_utils import run_bass_kernel_spmd

    nc = _get_program(debug_dump)
    res = run_bass_kernel_spmd(nc, in_maps, list(range(N_CORES)), trace=trace)
    return res.results, res.exec_time_ns


def kernel(**inputs):
    in_maps = _host_prep(**inputs)
    results, _ = run_on_device(in_maps, trace=False)
    out = np.concatenate([results[c]["out"].reshape(BL) for c in range(N_CORES)])
    return out.astype(np.float32)



# revision 6
# speedup vs baseline: 1.1716x; 1.1716x over previous
"""DeepFM (nn_DeepFM_77558519431939) Trainium2 Bass kernel, v2.

Strategy (8 NeuronCores, SPMD, no collectives):
  - Replicate the embedding tables on every core; data-parallel the batch
    (16384 samples -> 2048 per core).
  - The gather is descriptor-generation bound on the GpSimd SWDGE
    (~1.4us per 128-offset indirect instruction, ~8ns/row on every SWDGE
    path we measured), so gpsimd runs ONLY the 5 big fields:
      80 indirect_dma_start instructions (16 sample-blocks x 5 fields,
      128 rows each) against a bf16 augmented table tb [S, 12]
      (10 emb dims + w_first + pad).  No ucode library load.
  - Small fields 5-7 (vocab 10000/5000/1000) are computed on the idle
    TensorE via a two-level one-hot gather (exact):
      idx = 128*h + l.  Host uploads ohl [128, BL] one-hot of l, and
      per-group masks of h.  Table T' [128, G*120] has col
      g*120+hh*12+e = A[(g*10+hh)*128+l, e] (A = [V, 12] emb+wf+pad).
      Step A (PE):  G_g[(hh,e), s] = sum_l T'[l, g*120+(hh,e)] ohl[l, s]
      DVE:          M_g = G_g * mask_g        (mask = (h_s == g*10+hh))
      Step B (PE):  x_f[e, s] = sum_g P^T M_g  (P [120,12] tiled identity)
    All terms exact one-hot selections; fp32 PSUM holds single nonzero.
  - X [104, 2048] bf16, feature-major:
        rows f*12+e (e<10): emb dim e of field f; f*12+10: w_first;
        f*12+11: pad(0); 96..102: dense features; 103: const 1.0
  - Head per 128-sample block (weights folded on host, bf16 matmuls
    with fp32 PSUM accumulation):
        H1 = relu(W1s^T X)          (dense-proj + b1 folded into W1s)
        H2 = relu(W2^T H1 + b2)
        SD = SDw^T X                (rows 0..9 = s, 10..19 = dense_emb)
        XSQ = X[0:96]^2, SD2 = SD^2
        FIN = a1^T X + esq^T XSQ + es2^T SD2 + W3^T H2
        out = sigmoid(FIN)
"""

from contextlib import ExitStack

import numpy as np
import ml_dtypes

import concourse.bass as bass
import concourse.bacc as bacc
import concourse.mybir as mybir
import concourse.tile as tile

# ---- problem constants (hardcoded; must match the reference) ----
VOCABS = [1000000, 500000, 200000, 100000, 50000, 10000, 5000, 1000]
S = int(np.sum(VOCABS))  # 1,866,000
OFFSETS = np.concatenate([[0], np.cumsum(VOCABS)[:-1]]).astype(np.int64)
B = 16384
EMB = 10
N_DENSE = 7
F = len(VOCABS)      # 8
FB = 5               # big fields (indirect path)
FS = 3               # small fields (PE one-hot path)
HID = 128

N_CORES = 8
BL = B // N_CORES    # 2048 per core
RW = 12              # augmented row width (10 emb + wf + pad)
KX = 108             # X rows: 60 big + 4 pad + 36 small + 7 dense + 1 const
NBLK = BL // 128     # 16 sample blocks of 128
SL = 512             # small-field sample slice
NSL = BL // SL       # 4 slices
# one-hot groups: 10 h-values (120 (hh,e) rows) per group
GRP = [8, 4, 1]      # ceil(ceil(V/128)/10) for V=10000,5000,1000

F32 = mybir.dt.float32
BF16 = mybir.dt.bfloat16
I32 = mybir.dt.int32

_cached = {}


def _build_program(debug_dump=False):
    """Build the SPMD Bass program (same for all cores)."""
    nc = bacc.Bacc("TRN2", target_bir_lowering=False, debug=False)

    tb_d = nc.dram_tensor("tb", [S, RW], BF16, kind="ExternalInput").ap()
    idx_d = nc.dram_tensor("idxs", [128, NBLK * FB], I32,
                           kind="ExternalInput").ap()
    dn8_d = nc.dram_tensor("dn8", [8, BL], BF16, kind="ExternalInput").ap()
    # all small weights packed into one bf16 tensor:
    # cols: idn 0:128 | w1s 128:256 | w2 256:384 | sdw 385:405 |
    #       a1 405 | esq 406 | es2 407 | w3 408 | P 409:421
    wpk_d = nc.dram_tensor("wpk", [128, 517], BF16, kind="ExternalInput").ap()
    b2c_d = nc.dram_tensor("b2c", [128, 1], F32, kind="ExternalInput").ap()
    tp_d = [nc.dram_tensor(f"tp{q}", [128, GRP[q] * 120], BF16,
                           kind="ExternalInput").ap() for q in range(FS)]
    ohl_d = nc.dram_tensor("ohl", [128, FS * BL], BF16,
                           kind="ExternalInput").ap()
    msk_d = [nc.dram_tensor(f"msk{q}", [120, GRP[q] * BL], BF16,
                            kind="ExternalInput").ap() for q in range(FS)]
    out_d = nc.dram_tensor("out", [1, BL], F32, kind="ExternalOutput").ap()
    if debug_dump:
        xdmp_d = nc.dram_tensor("xdmp", [KX, BL], BF16,
                                kind="ExternalOutput").ap()

    with ExitStack() as ctx:
        tc = ctx.enter_context(tile.TileContext(nc))
        const = ctx.enter_context(tc.tile_pool(name="const", bufs=1))
        hpool = ctx.enter_context(tc.tile_pool(name="h", bufs=2))
        qpool = ctx.enter_context(tc.tile_pool(name="xsq", bufs=2))
        mpool = ctx.enter_context(tc.tile_pool(name="m", bufs=2))
        pp_g = ctx.enter_context(tc.tile_pool(name="ppg", bufs=2,
                                              space="PSUM"))
        pp_xf = ctx.enter_context(tc.tile_pool(name="ppxf", bufs=1,
                                               space="PSUM"))
        pp_h = ctx.enter_context(tc.tile_pool(name="pph", bufs=2,
                                              space="PSUM"))
        pp_m = ctx.enter_context(tc.tile_pool(name="ppm", bufs=1,
                                              space="PSUM"))

        # index tile first: the gathers depend only on it (sync queue)
        idx_t = const.tile([128, NBLK * FB], I32)
        nc.sync.dma_start(idx_t[:], idx_d[:])

        # non-gather-critical uploads go via the scalar HWDGE so the
        # gpsimd gather stream only waits on the sync-engine idx upload.
        wpk_t = const.tile([128, 517], BF16)
        nc.scalar.dma_start(wpk_t[:], wpk_d[:])
        b2_t = const.tile([128, 1], F32)
        nc.scalar.dma_start(b2_t[:], b2c_d[:])
        ohl_t = const.tile([128, FS * BL], BF16)
        nc.scalar.dma_start(ohl_t[:], ohl_d[:])
        tp_t = []
        for q in range(FS):
            t = const.tile([128, GRP[q] * 120], BF16, name=f"tp{q}")
            nc.scalar.dma_start(t[:], tp_d[q][:])
            tp_t.append(t)
        msk_t = []
        for q in range(FS):
            t = const.tile([120, GRP[q] * BL], BF16, name=f"msk{q}")
            nc.sync.dma_start(t[:], msk_d[q][:])
            msk_t.append(t)

        idn_t = wpk_t[:, 0:128]
        w1s_t = wpk_t[0:KX, 128:256]
        w2_t = wpk_t[:, 256:384]
        sdw_t = wpk_t[0:KX, 385:405]
        a1_t = wpk_t[0:KX, 405:406]
        esq_t = wpk_t[0:100, 406:407]
        es2_t = wpk_t[0:20, 407:408]
        w3_t = wpk_t[:, 408:409]
        p_t = [wpk_t[0:120, 409 + 36 * q:445 + 36 * q] for q in range(FS)]

        # X: feature-major activations (bf16)
        x_t = const.tile([KX, BL], BF16)
        nc.scalar.dma_start(x_t[100:108, :], dn8_d[:])
        nc.vector.memset(x_t[32:64, :], 0.0)

        out_sb = const.tile([1, BL], F32)

        RELU = mybir.ActivationFunctionType.Relu
        SQUARE = mybir.ActivationFunctionType.Square
        SIGMOID = mybir.ActivationFunctionType.Sigmoid

        # ---- gpsimd stream: 80 indirect gathers, nothing else ----
        gball = const.tile([128, NBLK, FB, RW], BF16)
        for j in range(NBLK):
            for f in range(FB):
                nc.gpsimd.indirect_dma_start(
                    out=gball[:, j, f, :],
                    out_offset=None,
                    in_=tb_d[:],
                    in_offset=bass.IndirectOffsetOnAxis(
                        ap=idx_t[:, j * FB + f:j * FB + f + 1], axis=0
                    ),
                )

        # ---- small fields via two-level one-hot on PE ----
        for c in range(NSL):
            cols = slice(SL * c, SL * (c + 1))
            m_ts = []
            for q in range(FS):
                G = GRP[q]
                m_t = mpool.tile([120, G * SL], BF16, tag=f"m{q}")
                for g in range(G):
                    gp = pp_g.tile([120, SL], F32, tag="g")
                    nc.tensor.matmul(
                        out=gp[:],
                        lhsT=tp_t[q][:, g * 120:(g + 1) * 120],
                        rhs=ohl_t[:, q * BL + SL * c:q * BL + SL * (c + 1)],
                        start=True, stop=True)
                    nc.vector.tensor_mul(
                        m_t[:, g * SL:(g + 1) * SL], gp[:],
                        msk_t[q][:, g * BL + SL * c:g * BL + SL * (c + 1)])
                m_ts.append(m_t)
            xfp = pp_xf.tile([100, SL], F32, tag="xf")
            xslc = xfp[64:100, :]
            nmm = sum(GRP)
            i = 0
            for q in range(FS):
                for g in range(GRP[q]):
                    nc.tensor.matmul(out=xslc, lhsT=p_t[q],
                                     rhs=m_ts[q][:, g * SL:(g + 1) * SL],
                                     start=(i == 0), stop=(i == nmm - 1))
                    i += 1
            nc.vector.tensor_copy(x_t[64:100, cols], xslc)

        # ---- per-block head ----
        def compute_block(j):
            cols = slice(128 * j, 128 * (j + 1))
            gb = gball[:, j, :, :]
            xp = pp_m.tile([60, 128], BF16, tag="xp")
            nc.tensor.transpose(out=xp[:], in_=gb[:], identity=idn_t)
            nc.vector.tensor_copy(x_t[0:60, cols], xp[:])

            # MLP
            h1p = pp_h.tile([HID, 128], F32, tag="hp")
            nc.tensor.matmul(out=h1p[:], lhsT=w1s_t, rhs=x_t[:, cols],
                             start=True, stop=True)
            h1_t = hpool.tile([HID, 128], BF16, tag="h")
            nc.scalar.activation(h1_t[:], h1p[:], RELU)
            h2p = pp_h.tile([HID, 128], F32, tag="hp")
            nc.tensor.matmul(out=h2p[:], lhsT=w2_t, rhs=h1_t[:],
                             start=True, stop=True)
            h2_t = hpool.tile([HID, 128], BF16, tag="h")
            nc.scalar.activation(h2_t[:], h2p[:], RELU, bias=b2_t)

            # s / dense_emb rows
            sdp = pp_m.tile([20, 128], F32, tag="sd")
            nc.tensor.matmul(out=sdp[:], lhsT=sdw_t, rhs=x_t[:, cols],
                             start=True, stop=True)

            xsq = qpool.tile([100, 128], BF16, tag="xsq")
            nc.vector.tensor_mul(xsq[:], x_t[0:100, cols], x_t[0:100, cols])
            sd2 = qpool.tile([20, 128], BF16, tag="sd2")
            nc.scalar.activation(sd2[:], sdp[:], SQUARE)

            # final accumulation + sigmoid
            fin = pp_m.tile([1, 128], F32, tag="fin")
            nc.tensor.matmul(out=fin[:], lhsT=a1_t, rhs=x_t[:, cols],
                             start=True, stop=False)
            nc.tensor.matmul(out=fin[:], lhsT=esq_t, rhs=xsq[:],
                             start=False, stop=False)
            nc.tensor.matmul(out=fin[:], lhsT=es2_t, rhs=sd2[:],
                             start=False, stop=False)
            nc.tensor.matmul(out=fin[:], lhsT=w3_t, rhs=h2_t[:],
                             start=False, stop=True)
            nc.scalar.activation(out_sb[:, cols], fin[:], SIGMOID)

        QB = NBLK // 4
        for k in range(4):
            for j in range(QB * k, QB * (k + 1)):
                compute_block(j)
            qc = slice((BL // 4) * k, (BL // 4) * (k + 1))
            nc.sync.dma_start(out_d[:, qc], out_sb[:, qc])
        if debug_dump:
            nc.sync.dma_start(xdmp_d[:], x_t[:])

    nc.compile()
    return nc


def _host_prep(sparse_feature, dense_feature, emb_table, W_dense, b_dense,
               w_first, b_first, W1, b1, W2, b2, W3, b3):
    """Build the augmented tables, folded weights, and per-core in_maps."""
    f32 = np.float32
    bf16 = ml_dtypes.bfloat16
    emb_table = np.asarray(emb_table, dtype=f32)
    W_dense = np.asarray(W_dense, dtype=f32)      # [10, 7]
    b_dense = np.asarray(b_dense, dtype=f32)      # [10]
    w_first = np.asarray(w_first, dtype=f32)      # [S+7]
    b_first = np.asarray(b_first, dtype=f32)      # [1]
    W1 = np.asarray(W1, dtype=f32)                # [90, 128]
    b1 = np.asarray(b1, dtype=f32)                # [128]
    W2 = np.asarray(W2, dtype=f32)                # [128, 128]
    b2 = np.asarray(b2, dtype=f32)                # [128]
    W3 = np.asarray(W3, dtype=f32)                # [128, 1]
    b3 = np.asarray(b3, dtype=f32)                # [1]

    tb = np.zeros((S, RW), dtype=bf16)
    tb[:, :EMB] = emb_table
    tb[:, EMB] = w_first[:S]

    # small-field one-hot tables T' [128, G*120]
    tps = []
    for q in range(FS):
        f = FB + q
        v = VOCABS[f]
        o = int(OFFSETS[f])
        G = GRP[q]
        A = np.zeros((G * 10 * 128, RW), dtype=f32)
        A[:v, :EMB] = emb_table[o:o + v]
        A[:v, EMB] = w_first[o:o + v]
        tp = np.ascontiguousarray(
            A.reshape(G * 10, 128, RW).transpose(1, 0, 2)
            .reshape(128, G * 10 * RW)).astype(bf16)
        tps.append(tp)

    def xrow(f):
        return f * RW if f < FB else 64 + (f - FB) * RW

    w1s = np.zeros((KX, HID), dtype=f32)
    for f in range(F):
        w1s[xrow(f):xrow(f) + EMB] = W1[f * EMB:(f + 1) * EMB]
    w1s[100:107] = W_dense.T @ W1[F * EMB:]              # [7,128]
    w1s[107] = b1 + b_dense @ W1[F * EMB:]

    sdw = np.zeros((KX, 20), dtype=f32)
    for f in range(F):
        for e in range(EMB):
            sdw[xrow(f) + e, e] = 1.0
    sdw[100:107, 0:10] = W_dense.T
    sdw[107, 0:10] = b_dense
    sdw[100:107, 10:20] = W_dense.T
    sdw[107, 10:20] = b_dense

    a1 = np.zeros((KX, 1), dtype=f32)
    for f in range(F):
        a1[xrow(f) + EMB] = 1.0
    a1[100:107, 0] = w_first[S:]
    a1[107] = b_first[0] + b3[0]

    esq = np.zeros((100, 1), dtype=f32)
    for f in range(F):
        esq[xrow(f):xrow(f) + EMB] = -0.5
    es2 = np.zeros((20, 1), dtype=f32)
    es2[0:10] = 0.5
    es2[10:20] = -0.5

    idx_g = (np.asarray(sparse_feature, dtype=np.int64)
             + OFFSETS[None, :]).astype(np.int64)          # [B, F]
    dense = np.asarray(dense_feature, dtype=f32)           # [B, 7]

    wpk = np.zeros((128, 517), dtype=bf16)
    wpk[:, 0:128] = np.eye(128, dtype=f32)
    wpk[0:KX, 128:256] = w1s
    wpk[:, 256:384] = W2
    wpk[0:KX, 385:405] = sdw
    wpk[0:KX, 405] = a1[:, 0]
    wpk[0:100, 406] = esq[:, 0]
    wpk[0:20, 407] = es2[:, 0]
    wpk[:, 408] = W3.reshape(HID)
    p12 = np.tile(np.eye(12, dtype=f32), (10, 1))        # [120, 12]
    for q in range(FS):
        wpk[0:120, 409 + 36 * q + 12 * q:409 + 36 * q + 12 * (q + 1)] = p12
    b2c = b2.reshape(128, 1).astype(f32)

    common = {"tb": tb, "wpk": wpk, "b2c": b2c}
    for q in range(FS):
        common[f"tp{q}"] = tps[q]

    in_maps = []
    lanes = np.arange(128)
    for c in range(N_CORES):
        lo, hi = c * BL, (c + 1) * BL
        # big fields: global ids, [128, NBLK*FB], col j*FB+f
        lg = idx_g[lo:hi, :FB].reshape(NBLK, 128, FB).astype(np.int32)
        idxs = np.ascontiguousarray(
            lg.transpose(1, 0, 2).reshape(128, NBLK * FB))
        # small fields: one-hot of low bits + per-group masks of high bits
        ohl = np.zeros((128, FS * BL), dtype=bf16)
        msks = []
        for q in range(FS):
            loc = np.asarray(sparse_feature[lo:hi, FB + q],
                             dtype=np.int64)
            il = loc % 128
            ih = loc // 128
            ohl[:, q * BL:(q + 1) * BL] = (lanes[:, None] == il[None, :])
            G = GRP[q]
            msk = np.zeros((120, G * BL), dtype=bf16)
            for g in range(G):
                eq = (ih[None, :] == (g * 10 + np.arange(10))[:, None])
                msk[:, g * BL:(g + 1) * BL] = np.repeat(eq, RW, axis=0)
            msks.append(msk)
        dn8 = np.ones((8, BL), dtype=bf16)
        dn8[:7] = dense[lo:hi].T
        im = dict(common, idxs=idxs, ohl=ohl, dn8=dn8)
        for q in range(FS):
            im[f"msk{q}"] = msks[q]
        in_maps.append(im)
    return in_maps


def _get_program(debug_dump=False):
    key = ("nc", debug_dump)
    if key not in _cached:
        _cached[key] = _build_program(debug_dump)
    return _cached[key]


def run_on_device(in_maps, trace=False, debug_dump=False):
    """Run the SPMD program on 8 NeuronCores.  Returns (results, exec_ns)."""
    from concourse.bass_utils import run_bass_kernel_spmd

    nc = _get_program(debug_dump)
    res = run_bass_kernel_spmd(nc, in_maps, list(range(N_CORES)), trace=trace)
    return res.results, res.exec_time_ns


def kernel(**inputs):
    in_maps = _host_prep(**inputs)
    results, _ = run_on_device(in_maps, trace=False)
    out = np.concatenate([results[c]["out"].reshape(BL) for c in range(N_CORES)])
    return out.astype(np.float32)


# revision 7
# speedup vs baseline: 1.2470x; 1.0644x over previous
"""DeepFM (nn_DeepFM_77558519431939) Trainium2 Bass kernel, v2.

Strategy (8 NeuronCores, SPMD, no collectives):
  - Replicate the embedding tables on every core; data-parallel the batch
    (16384 samples -> 2048 per core).
  - The gather is descriptor-generation bound on the GpSimd SWDGE
    (~1.4us per 128-offset indirect instruction, ~8ns/row on every SWDGE
    path we measured), so gpsimd runs ONLY the 5 big fields:
      80 indirect_dma_start instructions (16 sample-blocks x 5 fields,
      128 rows each) against a bf16 augmented table tb [S, 12]
      (10 emb dims + w_first + pad).  No ucode library load.
  - Small fields 5-7 (vocab 10000/5000/1000) are computed on the idle
    TensorE via a two-level one-hot gather (exact):
      idx = 128*h + l.  Host uploads ohl [128, BL] one-hot of l, and
      per-group masks of h.  Table T' [128, G*120] has col
      g*120+hh*12+e = A[(g*10+hh)*128+l, e] (A = [V, 12] emb+wf+pad).
      Step A (PE):  G_g[(hh,e), s] = sum_l T'[l, g*120+(hh,e)] ohl[l, s]
      DVE:          M_g = G_g * mask_g        (mask = (h_s == g*10+hh))
      Step B (PE):  x_f[e, s] = sum_g P^T M_g  (P [120,12] tiled identity)
    All terms exact one-hot selections; fp32 PSUM holds single nonzero.
  - X [104, 2048] bf16, feature-major:
        rows f*12+e (e<10): emb dim e of field f; f*12+10: w_first;
        f*12+11: pad(0); 96..102: dense features; 103: const 1.0
  - Head per 128-sample block (weights folded on host, bf16 matmuls
    with fp32 PSUM accumulation):
        H1 = relu(W1s^T X)          (dense-proj + b1 folded into W1s)
        H2 = relu(W2^T H1 + b2)
        SD = SDw^T X                (rows 0..9 = s, 10..19 = dense_emb)
        XSQ = X[0:96]^2, SD2 = SD^2
        FIN = a1^T X + esq^T XSQ + es2^T SD2 + W3^T H2
        out = sigmoid(FIN)
"""

from contextlib import ExitStack

import numpy as np
import ml_dtypes

import concourse.bass as bass
import concourse.bacc as bacc
import concourse.mybir as mybir
import concourse.tile as tile

# ---- problem constants (hardcoded; must match the reference) ----
VOCABS = [1000000, 500000, 200000, 100000, 50000, 10000, 5000, 1000]
S = int(np.sum(VOCABS))  # 1,866,000
OFFSETS = np.concatenate([[0], np.cumsum(VOCABS)[:-1]]).astype(np.int64)
B = 16384
EMB = 10
N_DENSE = 7
F = len(VOCABS)      # 8
FB = 5               # big fields (indirect path)
FS = 3               # small fields (PE one-hot path)
HID = 128

N_CORES = 8
BL = B // N_CORES    # 2048 per core
RW = 12              # augmented row width (10 emb + wf + pad)
KX = 108             # X rows: 60 big + 4 pad + 36 small + 7 dense + 1 const
NBLK = BL // 128     # 16 sample blocks of 128
SL = 512             # small-field sample slice
NSL = BL // SL       # 4 slices
# one-hot groups: 10 h-values (120 (hh,e) rows) per group
GRP = [8, 4, 1]      # ceil(ceil(V/128)/10) for V=10000,5000,1000

F32 = mybir.dt.float32
BF16 = mybir.dt.bfloat16
I32 = mybir.dt.int32

_cached = {}


def _build_program(debug_dump=False):
    """Build the SPMD Bass program (same for all cores)."""
    nc = bacc.Bacc("TRN2", target_bir_lowering=False, debug=False)

    tb_d = nc.dram_tensor("tb", [S, RW], BF16, kind="ExternalInput").ap()
    idx_d = nc.dram_tensor("idxs", [128, NBLK * FB], I32,
                           kind="ExternalInput").ap()
    dn8_d = nc.dram_tensor("dn8", [8, BL], BF16, kind="ExternalInput").ap()
    # all small weights packed into one bf16 tensor:
    # cols: idn 0:128 | w1s 128:256 | w2 256:384 | sdw 385:405 |
    #       a1 405 | esq 406 | es2 407 | w3 408 | P 409:421
    wpk_d = nc.dram_tensor("wpk", [128, 530], BF16, kind="ExternalInput").ap()
    b2c_d = nc.dram_tensor("b2c", [128, 1], F32, kind="ExternalInput").ap()
    tp_d = [nc.dram_tensor(f"tp{q}", [128, GRP[q] * 120], BF16,
                           kind="ExternalInput").ap() for q in range(FS)]
    ohl_d = nc.dram_tensor("ohl", [128, FS * BL], BF16,
                           kind="ExternalInput").ap()
    ihr_d = nc.dram_tensor("ihr", [128, FS * BL], BF16,
                           kind="ExternalInput").ap()
    out_d = nc.dram_tensor("out", [1, BL], F32, kind="ExternalOutput").ap()
    if debug_dump:
        xdmp_d = nc.dram_tensor("xdmp", [KX, BL], BF16,
                                kind="ExternalOutput").ap()

    with ExitStack() as ctx:
        tc = ctx.enter_context(tile.TileContext(nc))
        const = ctx.enter_context(tc.tile_pool(name="const", bufs=1))
        hpool = ctx.enter_context(tc.tile_pool(name="h", bufs=2))
        qpool = ctx.enter_context(tc.tile_pool(name="xsq", bufs=2))
        mpool = ctx.enter_context(tc.tile_pool(name="m", bufs=2))
        pp_g = ctx.enter_context(tc.tile_pool(name="ppg", bufs=2,
                                              space="PSUM"))
        pp_xf = ctx.enter_context(tc.tile_pool(name="ppxf", bufs=1,
                                               space="PSUM"))
        pp_h = ctx.enter_context(tc.tile_pool(name="pph", bufs=2,
                                              space="PSUM"))
        pp_m = ctx.enter_context(tc.tile_pool(name="ppm", bufs=1,
                                              space="PSUM"))

        # index tile first: the gathers depend only on it (sync queue)
        idx_t = const.tile([128, NBLK * FB], I32)
        nc.sync.dma_start(idx_t[:], idx_d[:])

        # non-gather-critical uploads go via the scalar HWDGE so the
        # gpsimd gather stream only waits on the sync-engine idx upload.
        wpk_t = const.tile([128, 530], BF16)
        nc.scalar.dma_start(wpk_t[:], wpk_d[:])
        b2_t = const.tile([128, 1], F32)
        nc.scalar.dma_start(b2_t[:], b2c_d[:])
        ohl_t = const.tile([128, FS * BL], BF16)
        nc.scalar.dma_start(ohl_t[:], ohl_d[:])
        tp_t = []
        for q in range(FS):
            t = const.tile([128, GRP[q] * 120], BF16, name=f"tp{q}")
            nc.scalar.dma_start(t[:], tp_d[q][:])
            tp_t.append(t)
        ihr_t = const.tile([128, FS * BL], BF16)
        nc.sync.dma_start(ihr_t[:], ihr_d[:])

        idn_t = wpk_t[:, 0:128]
        w1s_t = wpk_t[0:KX, 128:256]
        w2_t = wpk_t[:, 256:384]
        sdw_t = wpk_t[0:KX, 385:405]
        a1_t = wpk_t[0:KX, 405:406]
        esq_t = wpk_t[0:100, 406:407]
        es2_t = wpk_t[0:20, 407:408]
        w3_t = wpk_t[:, 408:409]
        p_t = [wpk_t[0:120, 409 + 36 * q:445 + 36 * q] for q in range(FS)]
        hv_c = 517

        # X: feature-major activations (bf16)
        x_t = const.tile([KX, BL], BF16)
        nc.scalar.dma_start(x_t[100:108, :], dn8_d[:])
        nc.vector.memset(x_t[32:64, :], 0.0)

        out_sb = const.tile([1, BL], F32)

        RELU = mybir.ActivationFunctionType.Relu
        SQUARE = mybir.ActivationFunctionType.Square
        SIGMOID = mybir.ActivationFunctionType.Sigmoid

        # ---- gpsimd stream: 80 indirect gathers, nothing else ----
        gball = const.tile([128, NBLK, FB, RW], BF16)
        for j in range(NBLK):
            for f in range(FB):
                nc.gpsimd.indirect_dma_start(
                    out=gball[:, j, f, :],
                    out_offset=None,
                    in_=tb_d[:],
                    in_offset=bass.IndirectOffsetOnAxis(
                        ap=idx_t[:, j * FB + f:j * FB + f + 1], axis=0
                    ),
                )

        # ---- build per-group h masks on DVE (exact bf16 compares) ----
        msk_t = []
        qg = 0
        EQ = mybir.AluOpType.is_equal
        for q in range(FS):
            G = GRP[q]
            t = const.tile([120, G * BL], BF16, name=f"msk{q}")
            for g in range(G):
                nc.vector.tensor_tensor(
                    out=t[:, g * BL:(g + 1) * BL],
                    in0=ihr_t[0:120, q * BL:(q + 1) * BL],
                    in1=wpk_t[0:120, hv_c + qg:hv_c + qg + 1].to_broadcast(
                        [120, BL]),
                    op=EQ)
                qg += 1
            msk_t.append(t)

        # ---- small fields via two-level one-hot on PE ----
        for c in range(NSL):
            cols = slice(SL * c, SL * (c + 1))
            m_ts = []
            for q in range(FS):
                G = GRP[q]
                m_t = mpool.tile([120, G * SL], BF16, tag=f"m{q}")
                for g in range(G):
                    gp = pp_g.tile([120, SL], F32, tag="g")
                    nc.tensor.matmul(
                        out=gp[:],
                        lhsT=tp_t[q][:, g * 120:(g + 1) * 120],
                        rhs=ohl_t[:, q * BL + SL * c:q * BL + SL * (c + 1)],
                        start=True, stop=True)
                    nc.vector.tensor_mul(
                        m_t[:, g * SL:(g + 1) * SL], gp[:],
                        msk_t[q][:, g * BL + SL * c:g * BL + SL * (c + 1)])
                m_ts.append(m_t)
            xfp = pp_xf.tile([100, SL], F32, tag="xf")
            xslc = xfp[64:100, :]
            nmm = sum(GRP)
            i = 0
            for q in range(FS):
                for g in range(GRP[q]):
                    nc.tensor.matmul(out=xslc, lhsT=p_t[q],
                                     rhs=m_ts[q][:, g * SL:(g + 1) * SL],
                                     start=(i == 0), stop=(i == nmm - 1))
                    i += 1
            nc.vector.tensor_copy(x_t[64:100, cols], xslc)

        # ---- per-block head ----
        def compute_block(j):
            cols = slice(128 * j, 128 * (j + 1))
            gb = gball[:, j, :, :]
            xp = pp_m.tile([60, 128], BF16, tag="xp")
            nc.tensor.transpose(out=xp[:], in_=gb[:], identity=idn_t)
            nc.vector.tensor_copy(x_t[0:60, cols], xp[:])

            # MLP
            h1p = pp_h.tile([HID, 128], F32, tag="hp")
            nc.tensor.matmul(out=h1p[:], lhsT=w1s_t, rhs=x_t[:, cols],
                             start=True, stop=True)
            h1_t = hpool.tile([HID, 128], BF16, tag="h")
            nc.scalar.activation(h1_t[:], h1p[:], RELU)
            h2p = pp_h.tile([HID, 128], F32, tag="hp")
            nc.tensor.matmul(out=h2p[:], lhsT=w2_t, rhs=h1_t[:],
                             start=True, stop=True)
            h2_t = hpool.tile([HID, 128], BF16, tag="h")
            nc.scalar.activation(h2_t[:], h2p[:], RELU, bias=b2_t)

            # s / dense_emb rows
            sdp = pp_m.tile([20, 128], F32, tag="sd")
            nc.tensor.matmul(out=sdp[:], lhsT=sdw_t, rhs=x_t[:, cols],
                             start=True, stop=True)

            xsq = qpool.tile([100, 128], BF16, tag="xsq")
            nc.vector.tensor_mul(xsq[:], x_t[0:100, cols], x_t[0:100, cols])
            sd2 = qpool.tile([20, 128], BF16, tag="sd2")
            nc.scalar.activation(sd2[:], sdp[:], SQUARE)

            # final accumulation + sigmoid
            fin = pp_m.tile([1, 128], F32, tag="fin")
            nc.tensor.matmul(out=fin[:], lhsT=a1_t, rhs=x_t[:, cols],
                             start=True, stop=False)
            nc.tensor.matmul(out=fin[:], lhsT=esq_t, rhs=xsq[:],
                             start=False, stop=False)
            nc.tensor.matmul(out=fin[:], lhsT=es2_t, rhs=sd2[:],
                             start=False, stop=False)
            nc.tensor.matmul(out=fin[:], lhsT=w3_t, rhs=h2_t[:],
                             start=False, stop=True)
            nc.scalar.activation(out_sb[:, cols], fin[:], SIGMOID)

        QB = NBLK // 4
        for k in range(4):
            for j in range(QB * k, QB * (k + 1)):
                compute_block(j)
            qc = slice((BL // 4) * k, (BL // 4) * (k + 1))
            nc.sync.dma_start(out_d[:, qc], out_sb[:, qc])
        if debug_dump:
            nc.sync.dma_start(xdmp_d[:], x_t[:])

    nc.compile()
    return nc


def _host_prep(sparse_feature, dense_feature, emb_table, W_dense, b_dense,
               w_first, b_first, W1, b1, W2, b2, W3, b3):
    """Build the augmented tables, folded weights, and per-core in_maps."""
    f32 = np.float32
    bf16 = ml_dtypes.bfloat16
    emb_table = np.asarray(emb_table, dtype=f32)
    W_dense = np.asarray(W_dense, dtype=f32)      # [10, 7]
    b_dense = np.asarray(b_dense, dtype=f32)      # [10]
    w_first = np.asarray(w_first, dtype=f32)      # [S+7]
    b_first = np.asarray(b_first, dtype=f32)      # [1]
    W1 = np.asarray(W1, dtype=f32)                # [90, 128]
    b1 = np.asarray(b1, dtype=f32)                # [128]
    W2 = np.asarray(W2, dtype=f32)                # [128, 128]
    b2 = np.asarray(b2, dtype=f32)                # [128]
    W3 = np.asarray(W3, dtype=f32)                # [128, 1]
    b3 = np.asarray(b3, dtype=f32)                # [1]

    tb = np.zeros((S, RW), dtype=bf16)
    tb[:, :EMB] = emb_table
    tb[:, EMB] = w_first[:S]

    # small-field one-hot tables T' [128, G*120]
    tps = []
    for q in range(FS):
        f = FB + q
        v = VOCABS[f]
        o = int(OFFSETS[f])
        G = GRP[q]
        A = np.zeros((G * 10 * 128, RW), dtype=f32)
        A[:v, :EMB] = emb_table[o:o + v]
        A[:v, EMB] = w_first[o:o + v]
        tp = np.ascontiguousarray(
            A.reshape(G * 10, 128, RW).transpose(1, 0, 2)
            .reshape(128, G * 10 * RW)).astype(bf16)
        tps.append(tp)

    def xrow(f):
        return f * RW if f < FB else 64 + (f - FB) * RW

    w1s = np.zeros((KX, HID), dtype=f32)
    for f in range(F):
        w1s[xrow(f):xrow(f) + EMB] = W1[f * EMB:(f + 1) * EMB]
    w1s[100:107] = W_dense.T @ W1[F * EMB:]              # [7,128]
    w1s[107] = b1 + b_dense @ W1[F * EMB:]

    sdw = np.zeros((KX, 20), dtype=f32)
    for f in range(F):
        for e in range(EMB):
            sdw[xrow(f) + e, e] = 1.0
    sdw[100:107, 0:10] = W_dense.T
    sdw[107, 0:10] = b_dense
    sdw[100:107, 10:20] = W_dense.T
    sdw[107, 10:20] = b_dense

    a1 = np.zeros((KX, 1), dtype=f32)
    for f in range(F):
        a1[xrow(f) + EMB] = 1.0
    a1[100:107, 0] = w_first[S:]
    a1[107] = b_first[0] + b3[0]

    esq = np.zeros((100, 1), dtype=f32)
    for f in range(F):
        esq[xrow(f):xrow(f) + EMB] = -0.5
    es2 = np.zeros((20, 1), dtype=f32)
    es2[0:10] = 0.5
    es2[10:20] = -0.5

    idx_g = (np.asarray(sparse_feature, dtype=np.int64)
             + OFFSETS[None, :]).astype(np.int64)          # [B, F]
    dense = np.asarray(dense_feature, dtype=f32)           # [B, 7]

    wpk = np.zeros((128, 530), dtype=bf16)
    wpk[:, 0:128] = np.eye(128, dtype=f32)
    wpk[0:KX, 128:256] = w1s
    wpk[:, 256:384] = W2
    wpk[0:KX, 385:405] = sdw
    wpk[0:KX, 405] = a1[:, 0]
    wpk[0:100, 406] = esq[:, 0]
    wpk[0:20, 407] = es2[:, 0]
    wpk[:, 408] = W3.reshape(HID)
    p12 = np.tile(np.eye(12, dtype=f32), (10, 1))        # [120, 12]
    for q in range(FS):
        wpk[0:120, 409 + 36 * q + 12 * q:409 + 36 * q + 12 * (q + 1)] = p12
    qg = 0
    for q in range(FS):
        for g in range(GRP[q]):
            wpk[0:120, 517 + qg] = np.repeat(g * 10 + np.arange(10), RW)
            qg += 1
    b2c = b2.reshape(128, 1).astype(f32)

    common = {"tb": tb, "wpk": wpk, "b2c": b2c}
    for q in range(FS):
        common[f"tp{q}"] = tps[q]

    in_maps = []
    lanes = np.arange(128)
    for c in range(N_CORES):
        lo, hi = c * BL, (c + 1) * BL
        # big fields: global ids, [128, NBLK*FB], col j*FB+f
        lg = idx_g[lo:hi, :FB].reshape(NBLK, 128, FB).astype(np.int32)
        idxs = np.ascontiguousarray(
            lg.transpose(1, 0, 2).reshape(128, NBLK * FB))
        # small fields: one-hot of low bits + replicated high bits
        ohl = np.zeros((128, FS * BL), dtype=bf16)
        ihr = np.zeros((128, FS * BL), dtype=bf16)
        for q in range(FS):
            loc = np.asarray(sparse_feature[lo:hi, FB + q],
                             dtype=np.int64)
            il = loc % 128
            ih = loc // 128
            ohl[:, q * BL:(q + 1) * BL] = (lanes[:, None] == il[None, :])
            ihr[:, q * BL:(q + 1) * BL] = ih[None, :].astype(bf16)
        dn8 = np.ones((8, BL), dtype=bf16)
        dn8[:7] = dense[lo:hi].T
        im = dict(common, idxs=idxs, ohl=ohl, ihr=ihr, dn8=dn8)
        in_maps.append(im)
    return in_maps


def _get_program(debug_dump=False):
    key = ("nc", debug_dump)
    if key not in _cached:
        _cached[key] = _build_program(debug_dump)
    return _cached[key]


def run_on_device(in_maps, trace=False, debug_dump=False):
    """Run the SPMD program on 8 NeuronCores.  Returns (results, exec_ns)."""
    from concourse.bass_utils import run_bass_kernel_spmd

    nc = _get_program(debug_dump)
    res = run_bass_kernel_spmd(nc, in_maps, list(range(N_CORES)), trace=trace)
    return res.results, res.exec_time_ns


def kernel(**inputs):
    in_maps = _host_prep(**inputs)
    results, _ = run_on_device(in_maps, trace=False)
    out = np.concatenate([results[c]["out"].reshape(BL) for c in range(N_CORES)])
    return out.astype(np.float32)
